# revision 1
# baseline (speedup 1.0000x reference)
"""Data-parallel GAT-module kernel for 8 Trainium2 NeuronCores.

Shards batch N=64 across the 8 cores (8 samples per core); A and all
1x1-conv weights are replicated (tiny). No cross-device communication in
the forward pass. Accepts FULL unsharded inputs, returns the FULL output.

Hardcoded problem shapes: x (64, 64, 256, 25), A (25, 25),
Wq/Wk (8, 64), Wv (64, 64), Wr (64, 8).
"""
import numpy as np
import jax
import jax.numpy as jnp
from functools import partial

N, C, T, V = 64, 64, 256, 25
H, O = 8, 64
N_CORES = 8

_fwd_cache = {}


def _forward(x, A, alpha, Wq, bq, Wk, bk, Wv, bv, Wr, br):
    # x: (N/8, C, T, V) local shard
    x_mean = x.mean(axis=2)                                           # (n, C, V)
    q = jnp.einsum('ncv,hc->nhv', x_mean, Wq) + bq[None, :, None]     # (n, H, V)
    k = jnp.einsum('ncv,hc->nhv', x_mean, Wk) + bk[None, :, None]     # (n, H, V)
    v = jnp.einsum('nctv,oc->notv', x, Wv) + bv[None, :, None, None]  # (n, O, T, V)
    attn = jnp.tanh(q[:, :, :, None] - k[:, :, None, :])              # (n, H, V, V)
    rep = jnp.einsum('nhuv,oh->nouv', attn, Wr) + br[None, :, None, None]
    masked = alpha * rep + A[None, None, :, :]                        # (n, O, V, V)
    out = jnp.einsum('ncuv,nctv->nctu', masked, v)                    # (n, O, T, V)
    return out


def _get_pmapped(n_dev):
    if n_dev not in _fwd_cache:
        _fwd_cache[n_dev] = jax.pmap(
            _forward,
            in_axes=(0, None, None, None, None, None, None, None, None, None, None),
            devices=jax.devices()[:n_dev],
        )
    return _fwd_cache[n_dev]


def kernel(x, A, alpha, Wq, bq, Wk, bk, Wv, bv, Wr, br):
    x = np.asarray(x, dtype=np.float32)
    n_dev = min(N_CORES, jax.device_count())
    per = N // n_dev
    xs = x.reshape(n_dev, per, C, T, V)
    f = _get_pmapped(n_dev)
    out = f(
        xs,
        jnp.asarray(A, jnp.float32),
        jnp.asarray(alpha, jnp.float32),
        jnp.asarray(Wq, jnp.float32),
        jnp.asarray(bq, jnp.float32),
        jnp.asarray(Wk, jnp.float32),
        jnp.asarray(bk, jnp.float32),
        jnp.asarray(Wv, jnp.float32),
        jnp.asarray(bv, jnp.float32),
        jnp.asarray(Wr, jnp.float32),
        jnp.asarray(br, jnp.float32),
    )
    out = np.asarray(out).reshape(N, O, T, V).astype(np.float32)
    return out



# revision 30
# speedup vs baseline: 9195.7492x; 9195.7492x over previous
"""GAT-module kernel for 8 Trainium2 NeuronCores (Bass/Tile).

Data-parallel: batch N=64 sharded 8 samples/core; weights replicated.
Math per sample (reference):
    x_mean = x.mean(t)                          (C, V)
    q = Wq @ x_mean + bq ; k = Wk @ x_mean + bk (H, V)
    attn[h,u,v] = tanh(q[h,u] - k[h,v])
    masked[o,u,v] = alpha*(Wr[o,:]@attn[:,u,v] + br[o]) + A[u,v]
    v_proj[o,t,v] = Wv[o,:]@x[:,t,v] + bv[o]
    out[o,t,u] = sum_v masked[o,u,v] * v_proj[o,t,v]

Device dataflow (bf16 compute, fp32 psum):
  GEMM1 per v: psum(O,T) = WvT.T @ x[:, :, v]; evict(+bv) -> v_proj v-major
  q/k: halving-tree sum over t (DVE) -> two small matmuls -> outer-diff
       (broadcast APs) -> tanh(+bq-bk) on ACT
  masked: matmul(alpha*WrT stationary, attn moving) in 2 free-chunks;
       eviction fuses +A (partition-broadcast) and +alpha*br (scalar)
  GEMM2 per o-group g (5 o's): stationary = block-diag((i,v),(i,u)) of
       masked rows (built by 5 tiny SBUF DMAs into a zeroed tile), moving =
       v_projT ((i,v),t) (one SBUF->SBUF re-partition DMA), psum(125,T)
       -> out DRAM as (n, o, u, t); host transposes to (n, o, t, u).
"""
import numpy as np
import ml_dtypes
from contextlib import ExitStack

import concourse.bass as bass
import concourse.bacc as bacc
import concourse.tile as tile
from concourse import mybir
from concourse.bass_utils import run_bass_kernel_spmd

N, C, T, V = 64, 64, 256, 25
H, O = 8, 64
N_CORES = 8
NL = N // N_CORES          # samples per core
GO = 5                     # o-channels per GEMM2 group
NG = (O + GO - 1) // GO    # 13 groups (12x5 + 1x4)
BD = GO * V                # 125 block-diag rows

BF16 = mybir.dt.bfloat16
F32 = mybir.dt.float32
AF = mybir.ActivationFunctionType

_cache = {}


def _build(race_detector=True, staircase=True):
    nc = bacc.Bacc()
    x_d = nc.declare_dram_parameter("x", [NL, C, T, V], BF16, isOutput=False)
    # wqt row C holds (bq-bk): paired with a ones-row in x_sum_ext
    wqt_d = nc.declare_dram_parameter("wqt", [C + 1, H], BF16, isOutput=False)
    wkt_d = nc.declare_dram_parameter("wkt", [C, H], BF16, isOutput=False)
    wvt_d = nc.declare_dram_parameter("wvt", [C, O], BF16, isOutput=False)
    wrt_d = nc.declare_dram_parameter("wrt", [H, O], BF16, isOutput=False)
    # a2[o, 25v+u] = A[u,v] + alpha*br[o]
    a2_d = nc.declare_dram_parameter("a2", [O, V * V], F32, isOutput=False)
    out_d = nc.declare_dram_parameter("out", [NL, O, V, T], BF16, isOutput=True)

    TV = T * V                     # 6400
    VV = V * V                     # 625
    MCH = [(0, 320), (320, VV)]    # masked free-chunks (<=512 fp32 psum)

    with tile.TileContext(nc) as tc, ExitStack() as ctx:
        # the staircase block-diag DMA trips a conservative false-positive in
        # the sim race detector (verified byte-exact standalone); allow
        # sim callers to disable it
        tc.race_detector_enabled = race_detector
        statics = ctx.enter_context(tc.tile_pool(name="statics", bufs=1))
        xp = ctx.enter_context(tc.tile_pool(name="xp", bufs=2))
        # one slot per sample: these tiles are read by many DMAs (13 rhsT /
        # 65 block-diag); slot reuse would make the next writer wait on all
        # 8 DMA-completion lanes and blow the per-instruction wait budget.
        vproj_p = ctx.enter_context(tc.tile_pool(name="vproj", bufs=NL))
        masked_p = ctx.enter_context(tc.tile_pool(name="masked", bufs=NL))
        work = ctx.enter_context(tc.tile_pool(name="work", bufs=2))
        rhs_p = ctx.enter_context(tc.tile_pool(name="rhs", bufs=3))
        bd_p = ctx.enter_context(tc.tile_pool(name="bd", bufs=3))
        osb_p = ctx.enter_context(tc.tile_pool(name="osb", bufs=3))
        psA = ctx.enter_context(tc.tile_pool(name="psA", bufs=2, space="PSUM"))
        psV = ctx.enter_context(tc.tile_pool(name="psV", bufs=4, space="PSUM"))
        psO = ctx.enter_context(tc.tile_pool(name="psO", bufs=2, space="PSUM"))

        wqt = statics.tile([C + 1, H], BF16)
        nc.sync.dma_start(out=wqt, in_=wqt_d[:, :])
        wkt = statics.tile([C, H], BF16)
        nc.sync.dma_start(out=wkt, in_=wkt_d[:, :])
        wvt = statics.tile([C, O], BF16)
        nc.sync.dma_start(out=wvt, in_=wvt_d[:, :])
        wrt = statics.tile([H, O], BF16)
        nc.sync.dma_start(out=wrt, in_=wrt_d[:, :])
        a2 = statics.tile([O, VV], F32)
        nc.sync.dma_start(out=a2, in_=a2_d[:, :])

        for n in range(NL):
            # ---- load x[n] as (C, (t,v)) contiguous ----
            x_sb = xp.tile([C, TV], BF16, tag="x")
            nc.sync.dma_start(out=x_sb, in_=x_d[n].rearrange("c t v -> c (t v)"))

            # ---- x_sum over t: halving tree on DVE (contiguous bf16) ----
            scr = work.tile([C, TV // 2], BF16, tag="scr")
            nc.vector.tensor_add(scr[:, : TV // 2], x_sb[:, : TV // 2],
                                 x_sb[:, TV // 2:])
            w = TV // 2
            while w > V * 2:
                nc.vector.tensor_add(scr[:, : w // 2], scr[:, : w // 2],
                                     scr[:, w // 2: w])
                w //= 2
            x_sum = work.tile([C + 1, V], BF16, tag="xsum")
            nc.vector.tensor_add(x_sum[:C, :], scr[:, :V], scr[:, V: 2 * V])
            nc.gpsimd.memset(x_sum[C: C + 1, :], 1.0)  # ones row pairs bias row

            # ---- q, k ----
            q_ps = psA.tile([H, V], F32, tag="ps_small")
            nc.tensor.matmul(q_ps, lhsT=wqt, rhs=x_sum, start=True, stop=True)
            k_ps = psA.tile([H, V], F32, tag="ps_small")
            nc.tensor.matmul(k_ps, lhsT=wkt, rhs=x_sum[:C, :], start=True, stop=True)
            qk_sb = work.tile([H, 2 * V], F32, tag="qk")
            nc.scalar.copy(qk_sb[:, :V], q_ps)
            nc.scalar.copy(qk_sb[:, V:], k_ps)

            # ---- attn2[h, (v,u)] = tanh(q[h,u] - k[h,v] + (bq-bk)[h]) ----
            d_sb = work.tile([H, VV], F32, tag="dsb")
            q_b = qk_sb[:, 0:V].unsqueeze(1).broadcast_to((H, V, V))
            k_b = qk_sb[:, V: 2 * V].unsqueeze(2).broadcast_to((H, V, V))
            nc.vector.tensor_sub(d_sb.rearrange("h (v u) -> h v u", v=V), q_b, k_b)
            attn = work.tile([H, VV], BF16, tag="attn")
            nc.scalar.activation(out=attn, in_=d_sb, func=AF.Tanh)

            # ---- masked2[o, (v,u)] = alpha*(Wr@attn + br) + A[u,v] ----
            masked_sb = masked_p.tile([O, 640], BF16, tag="masked")  # 640*2B=1280B: pitch==width for flat APs
            for (f0, f1) in MCH:
                m_ps = psA.tile([O, f1 - f0], F32, tag="ps_small")
                nc.tensor.matmul(m_ps, lhsT=wrt, rhs=attn[:, f0:f1],
                                 start=True, stop=True)
                nc.vector.tensor_add(masked_sb[:, f0:f1], m_ps, a2[:, f0:f1])

            # ---- GEMM1: v_proj[o, (v,t)] = WvT.T @ x + bv ----
            vproj = vproj_p.tile([O, TV], BF16, tag="vp")
            x_v = x_sb.rearrange("c (t v) -> c v t", v=V)
            for v in range(V):
                vp_ps = psV.tile([O, T], F32, tag="ps_v")
                nc.tensor.matmul(vp_ps, lhsT=wvt, rhs=x_v[:, v, :],
                                 start=True, stop=True)
                dst = vproj[:, v * T:(v + 1) * T]
                if v % 2 == 0:
                    nc.scalar.copy(out=dst, in_=vp_ps)
                else:
                    nc.vector.tensor_copy(out=dst, in_=vp_ps)

            # ---- GEMM2 per o-group ----
            vproj_v = vproj.rearrange("o (v t) -> o v t", v=V)
            for g in range(NG):
                go = min(GO, O - g * GO)
                rows = go * V
                rhsT = rhs_p.tile([BD, T], BF16, tag="rhsT")
                nc.sync.dma_start(out=rhsT[:rows, :],
                                  in_=vproj_v[g * GO: g * GO + go, :, :])
                # block-diag build: memset + 5 per-block DMAs land on up to 5
                # DMA-completion lanes; two partial DVE copies into a second
                # tile decouple that fan-in so the PE Ldweights waits on DVE
                # only (per-instruction sync-wait slots are scarce).
                bd_raw = bd_p.tile([BD, 128], BF16, tag="bd_raw")
                nc.gpsimd.memset(bd_raw, 0.0)
                for i in range(go):
                    o = g * GO + i
                    nc.scalar.dma_start(
                        out=bd_raw[i * V:(i + 1) * V, i * V:(i + 1) * V],
                        in_=masked_sb[o: o + 1, : VV].rearrange(
                            "p (v u) -> p v u", v=V))
                bd = bd_p.tile([BD, 128], BF16, tag="bd")
                nc.vector.tensor_copy(out=bd[:64, :], in_=bd_raw[:64, :])
                nc.vector.tensor_copy(out=bd[64:, :], in_=bd_raw[64:, :])
                o_ps = psO.tile([BD, T], F32, tag="ps_o")
                nc.tensor.matmul(o_ps[:rows, :], lhsT=bd[:rows, :rows],
                                 rhs=rhsT[:rows, :], start=True, stop=True)
                o_sb = osb_p.tile([BD, T], BF16, tag="osb")
                if g % 2 == 0:
                    nc.vector.tensor_copy(out=o_sb[:rows, :], in_=o_ps[:rows, :])
                else:
                    nc.scalar.copy(out=o_sb[:rows, :], in_=o_ps[:rows, :])
                nc.sync.dma_start(
                    out=out_d[n, g * GO: g * GO + go].rearrange("o u t -> (o u) t"),
                    in_=o_sb[:rows, :])
    nc.finalize()  # Bacc: legalizes multi-sem waits to the 1-wait ISA budget
    return nc


def _get_nc():
    if "nc" not in _cache:
        _cache["nc"] = _build()
    return _cache["nc"]


def _bf16(a):
    return np.asarray(a, dtype=np.float32).astype(ml_dtypes.bfloat16)


def _make_in_maps(x, A, alpha, Wq, bq, Wk, bk, Wv, bv, Wr, br):
    x = np.asarray(x, dtype=np.float32)
    alpha = np.float32(alpha)
    wqt = np.concatenate(
        [np.asarray(Wq, np.float32).T / T,
         (np.asarray(bq, np.float32) - np.asarray(bk, np.float32)).reshape(1, H)],
        axis=0)                                       # (C+1, H), mean + bias folded
    a2 = (np.asarray(A, np.float32).T.reshape(1, V * V)
          + (alpha * np.asarray(br, np.float32)).reshape(O, 1))  # (O, VV)
    consts = {
        "wqt": _bf16(wqt),
        "wkt": _bf16(np.asarray(Wk).T / T),
        "wvt": _bf16(np.asarray(Wv).T),               # (C,O)
        "wrt": _bf16(alpha * np.asarray(Wr).T),       # (H,O)
        "a2": np.ascontiguousarray(a2),
    }
    xb = x.astype(ml_dtypes.bfloat16)
    return [dict(consts, x=np.ascontiguousarray(xb[c * NL:(c + 1) * NL]))
            for c in range(N_CORES)]


def _assemble(results):
    # per-core out: (NL, O, V, T) bf16 -> full (N, O, T, V) f32
    parts = [r["out"] for r in results]
    full = np.concatenate(parts, axis=0).astype(np.float32)   # (N, O, V, T)
    return np.ascontiguousarray(full.transpose(0, 1, 3, 2))


def _bv_correction(inputs):
    """out += bv[o] * sum_v masked[n,o,u,v] (t-independent); bv is zero in
    the spec, so the device kernel omits it and this is a host no-op."""
    bv = np.asarray(inputs["bv"], np.float32)
    if not np.any(bv):
        return None
    x = np.asarray(inputs["x"], np.float32)
    alpha = np.float32(inputs["alpha"])
    Wq, bq = np.asarray(inputs["Wq"], np.float32), np.asarray(inputs["bq"], np.float32)
    Wk, bk = np.asarray(inputs["Wk"], np.float32), np.asarray(inputs["bk"], np.float32)
    Wr, br = np.asarray(inputs["Wr"], np.float32), np.asarray(inputs["br"], np.float32)
    A = np.asarray(inputs["A"], np.float32)
    xm = x.mean(axis=2)
    q = np.einsum("ncv,hc->nhv", xm, Wq) + bq[None, :, None]
    k = np.einsum("ncv,hc->nhv", xm, Wk) + bk[None, :, None]
    attn = np.tanh(q[:, :, :, None] - k[:, :, None, :])
    rep = np.einsum("nhuv,oh->nouv", attn, Wr) + br[None, :, None, None]
    masked = alpha * rep + A[None, None]
    return np.einsum("o,nouv->nou", bv, masked)  # (N, O, V=u)


def run(inputs, trace=False, **kw):
    nc = _get_nc()
    in_maps = _make_in_maps(**inputs)
    res = run_bass_kernel_spmd(nc, in_maps, list(range(N_CORES)),
                               trace=trace, **kw)
    out = _assemble(res.results)
    corr = _bv_correction(inputs)
    if corr is not None:
        out += corr[:, :, None, :]
    return out, res


def kernel(x, A, alpha, Wq, bq, Wk, bk, Wv, bv, Wr, br):
    out, _ = run(dict(x=x, A=A, alpha=alpha, Wq=Wq, bq=bq, Wk=Wk, bk=bk,
                      Wv=Wv, bv=bv, Wr=Wr, br=br))
    return out
